# revision 1
# baseline (speedup 1.0000x reference)
"""Trainium2 Bass kernel for nn_CrossAttentionBlock.

Per-core work (data-parallel over batch, core b handles batch element b):
  q = avgpool2(query); k = avgpool2(kv)                 (pool scale folded into weights)
  Q = Wq' @ q, K = Wk' @ k   ([o, s] layout, attn scale folded into Wq')
  V_T = k^T @ Wv'^T          ([s, o] layout -- produced pre-transposed)
  per head: S_T = K_h^T Q_h  ([k, q] layout), expS = exp(S_T) (no max-sub; scores are O(1))
            O_T = V_h^T^T expS (PE, accumulated over k-tiles)
            rowsum via ones-matmul (replicated across 32 partitions)
            O_n = O_T * 1/rowsum
  Y = Wo' @ O_n + b/16       (BN gamma/var + upsample 1/16 folded into Wo', beta into bias)
  out = upsample2x_bilinear(Y) + g*query   (separable 2x (3,1)/4 taps)

Schedule: scores for all 4 heads of a group issue as 4 row-tiled (32xK)
matmuls into two [128,1024] PSUM tiles; a single scalar-engine Exp per
tile; PV and rowsum matmuls are col-tiled.  The scalar engine (exp over
8.4M scores) is the critical path; DVE/GpSimd carry pooling, copies and
the upsample/identity tail underneath it.  All attention matmuls and
projections run in bf16 (PSUM accumulation fp32).
"""

import os
import sys

sys.path.insert(0, "/opt/trn_rl_repo")

import numpy as np
import ml_dtypes

import concourse.bass as bass
import concourse.tile as tile
from concourse import bacc, mybir
from concourse.bass_utils import run_bass_kernel_spmd

F32 = mybir.dt.float32
BF16 = mybir.dt.bfloat16
EPS = 1e-5
MULT = mybir.AluOpType.mult
ADD = mybir.AluOpType.add

C = 256          # channels
HW = 4096        # 64*64
S = 1024         # pooled spatial 32*32
NCORES = 8
KT = 8           # k tiles of 128 over S


def emit_kernel(tc, dram):
    nc = tc.nc
    from contextlib import ExitStack

    query_d, kv_d = dram["query"], dram["kv"]
    w_d = {n: dram[n] for n in ("wqt", "wkt", "wvt", "wot")}
    gvec_d, bvec_d = dram["gvec"], dram["bvec"]
    out_d = dram["out"]
    EXP = mybir.ActivationFunctionType.Exp

    with ExitStack() as ctx:
        consts = ctx.enter_context(tc.tile_pool(name="consts", bufs=1))
        qres = ctx.enter_context(tc.tile_pool(name="qres", bufs=1))
        kvbuf = ctx.enter_context(tc.tile_pool(name="kvbuf", bufs=4))
        poolw = ctx.enter_context(tc.tile_pool(name="poolw", bufs=2))
        pools = ctx.enter_context(tc.tile_pool(name="pools", bufs=1))

        # ---------------- input DMA ----------------
        # One serial DMA queue at ~350-420 GB/s: order by need.  kv/q half 0
        # gate the first attention rounds; weights are interleaved right
        # before their first consumer; everything for half 1 follows.
        q_tiles = [qres.tile([128, HW], F32, name=f"qres{g}", tag=f"qres{g}")
                   for g in range(2)]
        kv_raw = {}
        wsb = {}

        def dma_kv(half):
            for g in range(2):
                raw = kvbuf.tile([128, 2048], F32, tag="kvraw", name="kvraw")
                nc.sync.dma_start(
                    out=raw[:],
                    in_=kv_d[g * 128:(g + 1) * 128, half * 2048:(half + 1) * 2048])
                kv_raw[(g, half)] = raw

        def dma_q(half):
            for g in range(2):
                nc.sync.dma_start(
                    out=q_tiles[g][:, half * 2048:(half + 1) * 2048],
                    in_=query_d[g * 128:(g + 1) * 128, half * 2048:(half + 1) * 2048])

        def dma_w(name):
            tiles = []
            for g in range(2):
                t = consts.tile([128, 256], BF16, tag=f"w_{name}_{g}",
                                name=f"w_{name}_{g}")
                nc.sync.dma_start(out=t[:], in_=w_d[name][g * 128:(g + 1) * 128, :])
                tiles.append(t)
            wsb[name] = tiles

        dma_kv(0)
        dma_w("wkt")
        dma_w("wvt")
        dma_q(0)
        dma_w("wqt")
        dma_kv(1)
        dma_q(1)
        dma_w("wot")
        g_sb, b_sb = [], []
        for m in range(2):
            tg = consts.tile([128, 1], F32, tag=f"gv_{m}", name=f"gv_{m}")
            nc.sync.dma_start(out=tg[:], in_=gvec_d[m * 128:(m + 1) * 128, :])
            g_sb.append(tg)
            tb = consts.tile([128, 1], F32, tag=f"bv_{m}", name=f"bv_{m}")
            nc.sync.dma_start(out=tb[:], in_=bvec_d[m * 128:(m + 1) * 128, :])
            b_sb.append(tb)
        ones32 = consts.tile([128, 32], BF16, tag="ones32")
        nc.vector.memset(ones32[:], 1.0)

        # ---------------- 2x2 sum-pool (scale folded into weights) -------
        # k pooling on DVE (latency-critical), q pooling on GpSimd.
        def pool_half(eng, raw, dst_pool_view):
            # raw: [128, 2048] fp32 = 32 spatial rows (64 wide) -> dst [128,16,32]
            rawv = raw.rearrange("p (h w t) -> p h w t", h=32, w=32, t=2)
            pw = poolw.tile([128, 1024], F32, tag="pw")
            pwv = pw[:].rearrange("p (h w) -> p h w", h=32)
            eng.tensor_add(pwv, rawv[:, :, :, 0], rawv[:, :, :, 1])
            pw2 = pw[:].rearrange("p (h t w) -> p h t w", h=16, t=2, w=32)
            eng.tensor_add(dst_pool_view, pw2[:, :, 0, :], pw2[:, :, 1, :])

        q_pool = [pools.tile([128, S], BF16, name=f"qpool{g}", tag=f"qpool{g}")
                  for g in range(2)]
        k_pool = [pools.tile([128, S], BF16, name=f"kpool{g}", tag=f"kpool{g}")
                  for g in range(2)]
        def emit_pools(half):
            # all on DVE (~3x faster per op than GpSimd); half 1 is emitted
            # mid-loop so its DMA waits can't clog the DVE queue in the ramp.
            for g in range(2):
                kpv = k_pool[g][:].rearrange("p (h w) -> p h w", h=32)
                pool_half(nc.vector, kv_raw[(g, half)][:],
                          kpv[:, half * 16:(half + 1) * 16, :])
            for g in range(2):
                qpv = q_pool[g][:].rearrange("p (h w) -> p h w", h=32)
                pool_half(nc.vector,
                          q_tiles[g][:, half * 2048:(half + 1) * 2048],
                          qpv[:, half * 16:(half + 1) * 16, :])

        emit_pools(0)

        # ---------------- projections + attention ----------------
        qk_sb = ctx.enter_context(tc.tile_pool(name="qk_sb", bufs=1))
        vt_sbp = ctx.enter_context(tc.tile_pool(name="vt_sb", bufs=1))
        expp = ctx.enter_context(tc.tile_pool(name="expp", bufs=2))
        rcpp = ctx.enter_context(tc.tile_pool(name="rcpp", bufs=2))
        onp = ctx.enter_context(tc.tile_pool(name="onp", bufs=4))
        ysbp = ctx.enter_context(tc.tile_pool(name="ysbp", bufs=1))
        tup = ctx.enter_context(tc.tile_pool(name="tup", bufs=1))
        finp = ctx.enter_context(tc.tile_pool(name="finp", bufs=2))
        fin2p = ctx.enter_context(tc.tile_pool(name="fin2p", bufs=6))
        t3p = ctx.enter_context(tc.tile_pool(name="t3p", bufs=2))
        scrp = ctx.enter_context(tc.tile_pool(name="scrp", bufs=1))

        Q_slab = [qk_sb.tile([128, 1024], BF16, name=f"Qs{m}", tag=f"Qs{m}")
                  for m in range(2)]
        K_slab = [qk_sb.tile([128, 1024], BF16, name=f"Ks{m}", tag=f"Ks{m}")
                  for m in range(2)]
        vt_slab = [vt_sbp.tile([128, 1024], BF16, name=f"vt{h}", tag=f"vt{h}")
                   for h in range(2)]

        on_t = {}    # (g, qh) -> [128, 512] bf16 normalized attention out
        ysb = {}     # (m, qh) -> [128, 512] fp32 Y (pooled, scaled, + b/16)
        Tt = {}      # (m, half) -> [128, 1024] W-upsampled rows

        psS = ctx.enter_context(tc.tile_pool(name="psS", bufs=2, space="PSUM"))
        psOR = ctx.enter_context(tc.tile_pool(name="psOR", bufs=2, space="PSUM"))

        def proj_K(m, sh, pt):
            # K_slab[m][:, sh*512:] <- wkt[:,m]^T @ k_pool (spatial half sh)
            for g in range(2):
                nc.tensor.matmul(
                    pt[:, sh * 512:(sh + 1) * 512],
                    lhsT=wsb["wkt"][g][:, m * 128:(m + 1) * 128],
                    rhs=k_pool[g][:, sh * 512:(sh + 1) * 512],
                    start=(g == 0), stop=(g == 1),
                )
            nc.vector.tensor_copy(K_slab[m][:, sh * 512:(sh + 1) * 512],
                                  pt[:, sh * 512:(sh + 1) * 512])

        def proj_Q(m, nh, pt):
            for g in range(2):
                nc.tensor.matmul(
                    pt[:, nh * 512:(nh + 1) * 512],
                    lhsT=wsb["wqt"][g][:, m * 128:(m + 1) * 128],
                    rhs=q_pool[g][:, nh * 512:(nh + 1) * 512],
                    start=(g == 0), stop=(g == 1),
                )
            nc.vector.tensor_copy(Q_slab[m][:, nh * 512:(nh + 1) * 512],
                                  pt[:, nh * 512:(nh + 1) * 512])

        def proj_V(half, pt=None):
            # vt_slab[half]: 4 k-tiles b (128 kpos each) x 256 out-ch
            if pt is None:
                pt = psOR.tile([128, 1024], F32, tag="or", name=f"ptV{half}")
            for bq in range(4):
                b = half * 4 + bq
                for g in range(2):
                    nc.tensor.matmul(
                        pt[:, bq * 256:(bq + 1) * 256],
                        lhsT=k_pool[g][:, b * 128:(b + 1) * 128],
                        rhs=wsb["wvt"][g][:],
                        start=(g == 0), stop=(g == 1),
                    )
            nc.vector.tensor_copy(vt_slab[half][:], pt[:])

        # Software-pipelined attention: rounds are (chunk, k-tile); the
        # scores+exp of round r+1 are emitted before the PV/RS of round r so
        # the PE never stalls on the scalar engine.  Scores: 4 heads row-tiled
        # (4-way concurrent).  PV (natural col groups) and RS (swapped col
        # groups: head j -> group (j+2)%4) split into two 4-way slots, each
        # gated only on its own exp.
        CHUNKS = [(0, 0), (1, 0), (0, 1), (1, 1)]   # (g, qh)
        OR_tiles = {}
        eb_store = {}

        def emit_S_exp(ci, b):
            g, qh = CHUNKS[ci]
            ebs = []
            for pair in range(2):
                St = psS.tile([128, 1024], F32, tag="ps", name=f"S{ci}_{b}_{pair}")
                for jj in range(2):
                    j = 2 * pair + jj
                    nc.tensor.matmul(
                        St[:, jj * 512:(jj + 1) * 512],
                        lhsT=K_slab[g][32 * j:32 * j + 32, b * 128:(b + 1) * 128],
                        rhs=Q_slab[g][32 * j:32 * j + 32, qh * 512:(qh + 1) * 512],
                        start=True, stop=True,
                        tile_position=(32 * j, 0),
                    )
                eb = expp.tile([128, 1024], BF16, tag=f"e{pair}",
                               name=f"eb{ci}_{b}_{pair}")
                nc.scalar.activation(eb[:], St[:], EXP)
                ebs.append(eb)
            eb_store[(ci, b)] = ebs

        def emit_PVRS(ci, b):
            g, qh = CHUNKS[ci]
            if ci not in OR_tiles:
                OR_tiles[ci] = psOR.tile([128, 1024], F32, tag="or",
                                         name=f"OR{ci}")
            OR = OR_tiles[ci]
            OT, RS = OR[:, 0:512], OR[:, 512:1024]
            ebs = eb_store.pop((ci, b))
            for pair in range(2):           # slot: PV pair + RS pair, 4-way
                eb = ebs[pair]
                for jj in range(2):
                    j = 2 * pair + jj
                    h = 4 * g + j
                    nc.tensor.matmul(
                        OT[32 * j:32 * j + 32, :],
                        lhsT=vt_slab[b // 4][:, (b % 4) * 256 + 32 * h:
                                             (b % 4) * 256 + 32 * h + 32],
                        rhs=eb[:, jj * 512:(jj + 1) * 512],
                        start=(b == 0), stop=(b == KT - 1),
                        tile_position=(0, 32 * j),
                        skip_group_check=True,
                    )
                for jj in range(2):
                    j = 2 * pair + jj
                    js = (j + 2) % 4        # swapped col group for rowsums
                    nc.tensor.matmul(
                        RS[32 * js:32 * js + 32, :],
                        lhsT=ones32[:],
                        rhs=eb[:, jj * 512:(jj + 1) * 512],
                        start=(b == 0), stop=(b == KT - 1),
                        tile_position=(0, 32 * js),
                        skip_group_check=True,
                    )

        def finish_chunk(ci):
            g, qh = CHUNKS[ci]
            OR = OR_tiles.pop(ci)
            OT, RS = OR[:, 0:512], OR[:, 512:1024]
            rcp = rcpp.tile([128, 512], F32, tag="rcp", name=f"rcp{ci}")
            nc.vector.reciprocal_approx_fast(out=rcp[:], in_=RS)
            ot = onp.tile([128, 512], BF16, tag="on", name=f"on{ci}")
            nc.vector.tensor_mul(ot[0:64, :], OT[0:64, :], rcp[64:128, :])
            nc.vector.tensor_mul(ot[64:128, :], OT[64:128, :], rcp[0:64, :])
            on_t[(g, qh)] = ot

        wo_psum = {}

        def wo_proj_g(qh, g):
            # Y[m] += wot[g]^T @ on_t[(g,qh)]; g=0 right after its chunk so
            # only the g=1 half sits on the tail's critical path.
            if qh not in wo_psum:
                wo_psum[qh] = psOR.tile([128, 1024], F32, tag="or",
                                        name=f"Y{qh}")
            yp = wo_psum[qh]
            for m in range(2):
                nc.tensor.matmul(
                    yp[:, m * 512:(m + 1) * 512],
                    lhsT=wsb["wot"][g][:, m * 128:(m + 1) * 128],
                    rhs=on_t[(g, qh)][:],
                    start=(g == 0), stop=(g == 1),
                    skip_group_check=True,
                )

        def wo_finish(qh):
            yp = wo_psum.pop(qh)
            for m in range(2):
                st = ysbp.tile([128, 512], F32, tag=f"ysb{m}{qh}")
                if qh == 1:
                    # tail: the scalar engine is idle after the last exp
                    nc.scalar.add(st[:], yp[:, m * 512:(m + 1) * 512],
                                  b_sb[m][:])
                else:
                    nc.vector.tensor_scalar(st[:], yp[:, m * 512:(m + 1) * 512],
                                            b_sb[m][:], None, op0=ADD)
                ysb[(m, qh)] = st

        def w_upsample(m, half, eng):
            # [128,16h,32w] -> [128,16h,64] with taps (3,1)/(1,3), x4 edges
            y = ysb[(m, half)][:].rearrange("p (h w) -> p h w", h=16)
            tt = tup.tile([128, 1024], F32, tag=f"t{m}{half}")
            t4 = tt[:].rearrange("p (h w t) -> p h w t", h=16, w=32, t=2)
            eng.scalar_tensor_tensor(t4[:, :, 1:32, 0], y[:, :, 1:32], 3.0,
                                     y[:, :, 0:31], op0=MULT, op1=ADD)
            eng.tensor_scalar_mul(t4[:, :, 0, 0], y[:, :, 0], 4.0)
            eng.scalar_tensor_tensor(t4[:, :, 0:31, 1], y[:, :, 0:31], 3.0,
                                     y[:, :, 1:32], op0=MULT, op1=ADD)
            eng.tensor_scalar_mul(t4[:, :, 31, 1], y[:, :, 31], 4.0)
            Tt[(m, half)] = tt

        def h_upsample_body(m, half):
            # all rows of fin except the one cross-half boundary row.
            # m=0: fused STT on DVE; m=1: 3*Tt on DVE + adds on GpSimd
            # (TENSOR_TENSOR is the only elementwise Pool opcode).
            tc_t = Tt[(m, half)][:].rearrange("p (h x) -> p h x", h=16)
            fin = finp.tile([128, 2048], F32, tag="fin")
            f4 = fin[:].rearrange("p (h t x) -> p h t x", h=16, t=2, x=64)
            nc.vector.scalar_tensor_tensor(f4[:, 1:16, 0, :],
                                           tc_t[:, 1:16, :], 3.0,
                                           tc_t[:, 0:15, :],
                                           op0=MULT, op1=ADD)
            nc.vector.scalar_tensor_tensor(f4[:, 0:15, 1, :],
                                           tc_t[:, 0:15, :], 3.0,
                                           tc_t[:, 1:16, :],
                                           op0=MULT, op1=ADD)
            if half == 0:
                nc.vector.tensor_scalar_mul(f4[:, 0, 0, :], tc_t[:, 0, :], 4.0)
            else:
                nc.vector.tensor_scalar_mul(f4[:, 15, 1, :], tc_t[:, 15, :], 4.0)
            return fin

        def h_upsample_boundary(m, half, fin):
            # the one cross-half row; for half 0 (its fin buffer may already
            # be recycled by then) write into a scratch row instead.
            tc_t = Tt[(m, half)][:].rearrange("p (h x) -> p h x", h=16)
            if half == 0:
                row = scrp.tile([128, 64], F32, tag=f"brow{m}")
                tb = Tt[(m, 1)][:].rearrange("p (h x) -> p h x", h=16)
                nc.vector.scalar_tensor_tensor(row[:], tc_t[:, 15, :], 3.0,
                                               tb[:, 0, :], op0=MULT, op1=ADD)
                return row
            f4 = fin[:].rearrange("p (h t x) -> p h t x", h=16, t=2, x=64)
            ttop = Tt[(m, 0)][:].rearrange("p (h x) -> p h x", h=16)
            nc.vector.scalar_tensor_tensor(f4[:, 0, 0, :], tc_t[:, 0, :], 3.0,
                                           ttop[:, 15, :], op0=MULT, op1=ADD)
            return None

        aff1 = {}    # m -> [128, 2048] bf16 g*query for half 1 (prefolded)

        def final_out(m, half, fin_ap, c0, c1, addeng=None):
            # out = g*query + fin_ap (cols [c0:c1] of the half), then DMA.
            # half 1 uses the prefolded bf16 g*query; half 0 computes it here.
            fin2 = fin2p.tile([128, c1 - c0], F32, tag="fin2")
            if half == 1 and m in aff1:
                (addeng or nc.vector).tensor_add(fin2[:], aff1[m][:, c0:c1],
                                                 fin_ap)
            else:
                nc.vector.tensor_scalar(
                    fin2[:],
                    q_tiles[m][:, half * 2048 + c0:half * 2048 + c1],
                    g_sb[m][:], None, op0=MULT)
                (addeng or nc.vector).tensor_add(fin2[:], fin2[:], fin_ap)
            nc.sync.dma_start(
                out=out_d[m * 128:(m + 1) * 128,
                          half * 2048 + c0:half * 2048 + c1],
                in_=fin2[:])

        # ---------------- schedule ----------------
        def emit_prelude(ci):
            if ci == 0:
                # only what rounds b0..3 need (kv/q half 0); K sh1 and V h1
                # are deferred to emit_prelude0b so the first exp doesn't
                # wait on the full 8MB input load.
                ptK0 = psS.tile([128, 1024], F32, tag="ps", name="ptK0")
                proj_K(0, 0, ptK0)
                ptQ0 = psS.tile([128, 1024], F32, tag="ps", name="ptQ0")
                proj_Q(0, 0, ptQ0)
                proj_V(0)                      # psum from psOR
            elif ci == 1:
                ptK1 = psS.tile([128, 1024], F32, tag="ps", name="ptK1")
                proj_K(1, 0, ptK1)
                proj_K(1, 1, ptK1)
                ptQ1 = psS.tile([128, 1024], F32, tag="ps", name="ptQ1")
                proj_Q(1, 0, ptQ1)
            elif ci == 2:
                ptQ0b = psS.tile([128, 1024], F32, tag="ps", name="ptQ0b")
                proj_Q(0, 1, ptQ0b)
            else:
                ptQ1b = psS.tile([128, 1024], F32, tag="ps", name="ptQ1b")
                proj_Q(1, 1, ptQ1b)

        def emit_prelude0b():
            # second half of chunk-0's inputs (kv/q half 1)
            ptK0b = psS.tile([128, 1024], F32, tag="ps", name="ptK0b")
            proj_K(0, 1, ptK0b)
            ptV1 = psS.tile([128, 1024], F32, tag="ps", name="ptV1")
            proj_V(1, ptV1)

        def emit_outputs(half):
            # everything downstream of wo_proj(half); engine split by m,
            # final adds alternate engines.
            for m in range(2):
                w_upsample(m, half, nc.vector)
            fins = [h_upsample_body(m, half) for m in range(2)]
            if half == 1:
                for m in range(2):
                    h_upsample_boundary(m, 1, fins[m])
            if half == 0:
                final_out(0, 0, fins[0][:, 0:1024], 0, 1024, nc.vector)
                final_out(0, 0, fins[0][:, 1024:1984], 1024, 1984, nc.gpsimd)
                final_out(1, 0, fins[1][:, 0:1024], 0, 1024, nc.gpsimd)
                final_out(1, 0, fins[1][:, 1024:1984], 1024, 1984, nc.vector)
            else:
                # tail: GpSimd adds are 3-4x slower than DVE -- keep the
                # whole chain on DVE, 512-col chunks so DMA drains early
                for m in range(2):
                    for ck in range(4):
                        c0, c1 = ck * 512, (ck + 1) * 512
                        final_out(m, 1, fins[m][:, c0:c1], c0, c1, nc.vector)

        ROUNDS = [(ci, b) for ci in range(4) for b in range(KT)]
        emit_prelude(0)
        emit_S_exp(0, 0)
        for r in range(len(ROUNDS)):
            ci, b = ROUNDS[r]
            if r + 1 < len(ROUNDS):
                ci2, b2 = ROUNDS[r + 1]
                if (ci2, b2) == (0, 1):
                    emit_pools(1)
                elif (ci2, b2) == (0, 4):
                    emit_prelude0b()
                    emit_prelude(1)
                elif (ci2, b2) == (1, 0):
                    emit_prelude(2)
                    emit_prelude(3)
                    for m in range(2):   # prefold g*query for the half-1 tail
                        a = fin2p.tile([128, 2048], BF16, tag=f"aff1_{m}",
                                       name=f"aff1_{m}", bufs=1)
                        nc.vector.tensor_scalar(
                            a[:], q_tiles[m][:, 2048:4096],
                            g_sb[m][:], None, op0=MULT)
                        aff1[m] = a
                emit_S_exp(ci2, b2)
            emit_PVRS(ci, b)
            if b == KT - 1:
                finish_chunk(ci)
                g, qh = CHUNKS[ci]
                wo_proj_g(qh, g)
                if g == 1:
                    wo_finish(qh)
                    emit_outputs(qh)
        # half-0 cross boundary row (needs Tt[(m,1)]) via scratch
        row0 = h_upsample_boundary(0, 0, None)
        row1 = h_upsample_boundary(1, 0, None)
        final_out(0, 0, row0[:], 1984, 2048)
        final_out(1, 0, row1[:], 1984, 2048)


def build_module(n_iters=1):
    nc = bacc.Bacc(
        "TRN2",
        target_bir_lowering=False,
        debug=False,
        enable_asserts=False,
    )
    dram = {}
    dram["query"] = nc.dram_tensor("query", [C, HW], F32, kind="ExternalInput").ap()
    dram["kv"] = nc.dram_tensor("kv", [C, HW], F32, kind="ExternalInput").ap()
    for n in ("wqt", "wkt", "wvt", "wot"):
        dram[n] = nc.dram_tensor(n, [C, C], BF16, kind="ExternalInput").ap()
    dram["gvec"] = nc.dram_tensor("gvec", [C, 1], F32, kind="ExternalInput").ap()
    dram["bvec"] = nc.dram_tensor("bvec", [C, 1], F32, kind="ExternalInput").ap()
    dram["out"] = nc.dram_tensor("out", [C, HW], F32, kind="ExternalOutput").ap()

    with tile.TileContext(nc) as tc:
        if n_iters == 1:
            emit_kernel(tc, dram)
        else:
            with tc.For_i(0, n_iters, 1):
                emit_kernel(tc, dram)
    nc.compile()
    return nc


_NC_CACHE = {}


def _get_module(n_iters=1):
    if n_iters not in _NC_CACHE:
        _NC_CACHE[n_iters] = build_module(n_iters)
    return _NC_CACHE[n_iters]


def fold_weights(Wq, Wk, Wv, Wo, bn_gamma, bn_beta, bn_mean, bn_var, num_heads):
    nh = int(num_heads)
    hd = C // nh
    scale = np.float32(hd ** -0.5)
    wqt = np.ascontiguousarray((0.25 * scale * Wq).T.astype(ml_dtypes.bfloat16))
    wkt = np.ascontiguousarray((0.25 * Wk).T.astype(ml_dtypes.bfloat16))
    wvt = np.ascontiguousarray((0.25 * Wv).T.astype(ml_dtypes.bfloat16))
    inv = 1.0 / np.sqrt(bn_var.astype(np.float32) + EPS)
    g = (bn_gamma * inv).astype(np.float32)
    bb = (bn_beta - bn_mean * bn_gamma * inv).astype(np.float32)
    wot = np.ascontiguousarray(((g[:, None] * Wo) / 16.0).T.astype(ml_dtypes.bfloat16))
    return wqt, wkt, wvt, wot, g, bb / 16.0


LAST_RESULTS = None


def kernel(query, kv, Wq, Wk, Wv, Wo, bn_gamma, bn_beta, bn_mean, bn_var, num_heads):
    global LAST_RESULTS
    query = np.asarray(query, dtype=np.float32)
    kv = np.asarray(kv, dtype=np.float32)
    assert int(num_heads) == 8 and query.shape == (NCORES, C, 64, 64)

    wqt, wkt, wvt, wot, g, bb16 = fold_weights(
        np.asarray(Wq, np.float32), np.asarray(Wk, np.float32),
        np.asarray(Wv, np.float32), np.asarray(Wo, np.float32),
        np.asarray(bn_gamma, np.float32), np.asarray(bn_beta, np.float32),
        np.asarray(bn_mean, np.float32), np.asarray(bn_var, np.float32),
        num_heads,
    )
    shared = {
        "wqt": wqt, "wkt": wkt, "wvt": wvt, "wot": wot,
        "gvec": np.ascontiguousarray(g.reshape(C, 1)),
        "bvec": np.ascontiguousarray(bb16.reshape(C, 1)),
    }
    in_maps = []
    for b in range(NCORES):
        m = dict(shared)
        m["query"] = np.ascontiguousarray(query[b].reshape(C, HW))
        m["kv"] = np.ascontiguousarray(kv[b].reshape(C, HW))
        in_maps.append(m)

    nc = _get_module(int(os.environ.get("KERNEL_ITERS", "1")))
    res = run_bass_kernel_spmd(nc, in_maps, list(range(NCORES)))
    LAST_RESULTS = res
    out = np.stack([res.results[b]["out"].reshape(C, 64, 64) for b in range(NCORES)])
    return out.astype(np.float32)

